# revision 31
# baseline (speedup 1.0000x reference)
"""Trainium2 Bass kernel for nn_BlurLayer (B=128, 224x224x3, per-sample
rotated-line motion blur, SAME depthwise conv).

Self-contained: kernel(**inputs) -> np.ndarray. Shards the batch over 8
NeuronCores (pure data parallel: 16 samples per core), compiles + runs one
SPMD Bass program via concourse.bass_utils.run_bass_kernel_spmd, gathers
the full output.

Method: the rotated blur kernel's taps (all equal 1/size) are grouped by
kernel column; each column group becomes a banded 0/1 weight matrix
contracted over image rows on the PE (PSUM-accumulated bf16 matmuls), and
the horizontal offset between groups is a static +3-elem slide of the
moving operand. All per-sample alignment (base column, transpose, flip,
integer shear) is baked into host-side data placement, so the device
program is fully static with no dynamic registers and no on-device shift
copies. Diagonal-ish lines are sheared by one column per row on the host
(making them near-vertical, shrinking the group count); sheared outputs
are written in sheared coordinates (wider rows) and unsheared on the
host. The 1/size scale is folded into the image pixels on the host.
Images/weights/outputs move as bf16/fp8/bf16 to cut HBM traffic; PSUM
accumulates in fp32. Slots are packed by a local search minimizing
sum-over-slots of (psum width x max group count).
"""

import math

import numpy as np

import concourse.mybir as mybir

MAXK = 32
H = W = 224
C = 3

WQ_UN = 224            # psum width in pixels per row-half, unsheared
WQ_SH = 335            # psum width in pixels per row-half, sheared (k=1)
W_UN = WQ_UN * C       # 672 elems
W_SH = WQ_SH * C       # 1005 elems
CHUNK0 = 512           # first psum chunk width (one full bank of fp32)

N_CORES = 8
SLOTS = 16


def split_sync_waits(nc, max_waits=1):
    n_split = 0
    for fn in nc.m.functions:
        for blk in fn.blocks:
            new_insts = []
            for inst in blk.instructions:
                si = inst.sync_info
                waits = list(si.on_wait) if (si and si.on_wait) else []
                if len(waits) > max_waits:
                    keep = waits[-max_waits:]
                    extra = waits[:-max_waits]
                    for j, w in enumerate(extra):
                        n_split += 1
                        nop = mybir.InstNoOp(
                            name=f"{inst.name}-waitsplit-{j}",
                            engine=inst.engine,
                            ins=[], outs=[],
                            sync_info=mybir.SyncInfo(on_wait=[w], on_update=[]),
                        )
                        new_insts.append(nop)
                    inst.sync_info = mybir.SyncInfo(on_wait=keep,
                                                    on_update=list(si.on_update or []))
                new_insts.append(inst)
            blk.instructions = new_insts
    return n_split


# ---------------------------------------------------------------- host math
def rotate_nearest_np(img, rad):
    K = img.shape[0]
    cos, sin = np.cos(rad), np.sin(rad)
    coords = np.arange(K, dtype=np.float32)
    yy, xx = np.meshgrid(coords, coords, indexing="ij")
    e = np.float32(K - 1)
    x_off = (e - (cos * e - sin * e)) * 0.5
    y_off = (e - (sin * e + cos * e)) * 0.5
    sx = cos * xx - sin * yy + x_off
    sy = sin * xx + cos * yy + y_off
    ix = np.round(sx).astype(np.int32)
    iy = np.round(sy).astype(np.int32)
    valid = (ix >= 0) & (ix < K) & (iy >= 0) & (iy < K)
    g = img[np.clip(iy, 0, K - 1), np.clip(ix, 0, K - 1)]
    return np.where(valid, g, np.float32(0.0))


def sample_taps(tbl_ch0, amt_b, ang_b):
    """-> (scale, ys, xs): tap rows/cols of the rotated kernel."""
    rad = np.float32(ang_b * math.pi / 180.0)
    ker = rotate_nearest_np(tbl_ch0[amt_b], rad)
    ys, xs = np.nonzero(ker)
    scale = float(ker[ys[0], xs[0]])
    return np.float32(scale), ys.astype(np.int64), xs.astype(np.int64)


def _span(v):
    return int(v.max() - v.min() + 1)


def span_options(ys, xs):
    """(span_unsheared, span_sheared): unsheared picks transpose; sheared
    picks k=+1 with optional flip (k=-1 equivalent)."""
    s_un = min(_span(xs), _span(ys))
    s_sh = min(_span(xs - ys), _span(xs + ys))
    return s_un, s_sh


def pack_slots(spans_un, spans_sh):
    """Partition 128 samples into 16 slots of 8 and pick slot modes
    (unsheared width 672 / sheared width 1005) minimizing
    sum(2 * width * max-span).  Local search with swaps + mode flips."""
    B = len(spans_un)
    n_slots = B // N_CORES

    def span_of(b, sh):
        # a sheared-width slot can host any sample (per-half placement is
        # free-form), so it gets the best of both spans
        return min(spans_sh[b], spans_un[b]) if sh else spans_un[b]

    def slot_cost(members, sh):
        w = W_SH if sh else W_UN
        return 2 * w * max(span_of(b, sh) for b in members)

    # init: solo-preference pools, sorted desc, chunked
    pref_sh = [b for b in range(B) if W_SH * spans_sh[b] < W_UN * spans_un[b]]
    pref_un = [b for b in range(B) if b not in pref_sh]
    pref_sh.sort(key=lambda b: -spans_sh[b])
    while len(pref_sh) % N_CORES:
        pref_un.append(pref_sh.pop())
    pref_un.sort(key=lambda b: -spans_un[b])
    slots = [pref_sh[i:i + N_CORES] for i in range(0, len(pref_sh), N_CORES)]
    modes = [True] * len(slots)
    slots += [pref_un[i:i + N_CORES] for i in range(0, len(pref_un), N_CORES)]
    modes += [False] * (n_slots - len(modes))

    costs = [slot_cost(m, s) for m, s in zip(slots, modes)]
    improved = True
    rounds = 0
    while improved and rounds < 60:
        improved = False
        rounds += 1
        for i in range(n_slots):
            for s in (True, False):
                if modes[i] != s:
                    c2 = slot_cost(slots[i], s)
                    if c2 < costs[i]:
                        modes[i], costs[i] = s, c2
                        improved = True
            for j in range(i + 1, n_slots):
                for a in range(N_CORES):
                    for b in range(N_CORES):
                        slots[i][a], slots[j][b] = slots[j][b], slots[i][a]
                        ci = slot_cost(slots[i], modes[i])
                        cj = slot_cost(slots[j], modes[j])
                        if ci + cj < costs[i] + costs[j]:
                            costs[i], costs[j] = ci, cj
                            improved = True
                        else:
                            slots[i][a], slots[j][b] = slots[j][b], slots[i][a]
    return slots, modes


def plan(kernels_table, amt, angles):
    """Full host plan: per-sample variants + slot schedule."""
    B = len(amt)
    tbl_ch0 = np.ascontiguousarray(kernels_table[:, :, :, 0])
    scales, taps = [], []
    for b in range(B):
        s, ys, xs = sample_taps(tbl_ch0, int(amt[b]), int(angles[b]))
        scales.append(s)
        taps.append((ys, xs))
    spans_un = [span_options(*taps[b])[0] for b in range(B)]
    spans_sh = [span_options(*taps[b])[1] for b in range(B)]

    slots, modes = pack_slots(spans_un, spans_sh)

    # order slots: interleave light/heavy by PE cost (light first)
    def pe_cost(i):
        w = W_SH if modes[i] else W_UN
        sp = [min(spans_sh[b], spans_un[b]) if modes[i] else spans_un[b]
              for b in slots[i]]
        return 2 * w * max(sp)
    order = sorted(range(len(slots)), key=pe_cost)
    ileave = []
    lo, hi = 0, len(order) - 1
    while lo <= hi:
        ileave.append(order[lo])
        if lo != hi:
            ileave.append(order[hi])
        lo += 1
        hi -= 1
    slots = [slots[i] for i in ileave]
    modes = [modes[i] for i in ileave]

    # per-sample final variant given slot mode.  In a sheared-width slot a
    # sample may use k in {-1, 0, +1}; the per-half placement constant D_h
    # absorbs everything (E_h below is the psum-column origin per half:
    # out[r, c] = dev[r, q] with q = c - k*r - E_h).
    samples = {}
    for j, (members, sh) in enumerate(zip(slots, modes)):
        for c, b in enumerate(members):
            ys, xs = taps[b]
            cand = [(min(_span(xs), _span(ys)), 0,
                     _span(ys) < _span(xs))]
            if sh:
                cand.append((_span(xs - ys), 1, False))
                cand.append((_span(xs + ys), -1, False))
            cand.sort(key=lambda t: (t[0], abs(t[1])))
            _, k, tr = cand[0]
            ky, kx = (xs, ys) if tr else (ys, xs)
            kxp = kx - k * ky
            m = int(kxp.min())
            G = int(kxp.max()) - m + 1
            if k == 1:
                E = (-111, -223)
            elif k == -1:
                E = (0, 112)
            else:
                E = (0, 0)
            D = tuple(15 - 15 * k - m - e for e in E)
            samples[b] = dict(slot=j, core=c, tr=bool(tr), k=k,
                              m=m, G=G, D=D, E=E, ky=ky, kxp=kxp,
                              scale=scales[b])

    gmax = []
    for j, (members, sh) in enumerate(zip(slots, modes)):
        gmax.append(max(samples[b]["G"] for b in members))

    meta = dict(slots=slots, modes=modes, gmax=gmax, samples=samples)

    # static widths
    sh_idx = [j for j in range(len(slots)) if modes[j]]
    un_idx = [j for j in range(len(slots)) if not modes[j]]
    meta["sh_idx"] = sh_idx
    meta["un_idx"] = un_idx
    meta["col_base"] = np.concatenate([[0], np.cumsum([g * 224 for g in gmax])])[:-1]
    meta["totcols"] = int(sum(g * 224 for g in gmax))
    return meta


# ------------------------------------------------------------- host tensors
def prepare_host(x, kernels_table, amt, angles, n_cores=N_CORES):
    import ml_dtypes

    B = x.shape[0]
    meta = plan(kernels_table, amt, angles)
    slots, modes, gmax = meta["slots"], meta["modes"], meta["gmax"]
    samples = meta["samples"]
    n_sh, n_un = len(meta["sh_idx"]), len(meta["un_idx"])
    # slot j -> index within its dram tensor
    slot_sub = {}
    for i, j in enumerate(meta["sh_idx"]):
        slot_sub[j] = i
    for i, j in enumerate(meta["un_idx"]):
        slot_sub[j] = i
    meta["slot_sub"] = slot_sub
    # device image layout: per slot [2, 128, TW] (row-halves materialized,
    # per-half column offset pre-applied)
    TWmax_sh = 3 * (max([gmax[j] for j in meta["sh_idx"]], default=1) - 1) + W_SH
    TWmax_un = 3 * (max([gmax[j] for j in meta["un_idx"]], default=1) - 1) + W_UN
    meta["TWmax_sh"], meta["TWmax_un"] = TWmax_sh, TWmax_un

    in_maps = []
    for c in range(n_cores):
        ximg_sh = np.zeros((max(n_sh, 1), 2, 128, TWmax_sh), ml_dtypes.bfloat16)
        ximg_un = np.zeros((max(n_un, 1), 2, 128, TWmax_un), ml_dtypes.bfloat16)
        wt = np.zeros((128, meta["totcols"]), np.uint8)  # fp8e4 bit pattern
        for j in range(len(slots)):
            b = slots[j][c]
            sp = samples[b]
            G, D, k = sp["G"], sp["D"], sp["k"]
            # variant image: scaled, maybe transposed
            img = x[b].astype(np.float32) * sp["scale"]
            if sp["tr"]:
                img = img.transpose(1, 0, 2)
            img = np.ascontiguousarray(img).reshape(H, W * C).astype(
                ml_dtypes.bfloat16)
            sh = modes[j]
            TW = 3 * (gmax[j] - 1) + (W_SH if sh else W_UN)
            dst = ximg_sh[slot_sub[j]] if sh else ximg_un[slot_sub[j]]
            # dst[h, p, 3u'+ch] = img[y(h,p), 3*(u' + k*y - D[h])+ch]
            for h, y0 in ((0, 0), (1, 96)):
                if k == 0:
                    lo = 3 * D[h]
                    s0, s1 = max(0, lo), min(TW, lo + W * C)
                    if s1 > s0:
                        dst[h, :, s0:s1] = img[y0:y0 + 128, s0 - lo:s1 - lo]
                else:
                    for p in range(128):
                        y = y0 + p
                        lo = 3 * (D[h] - k * y)
                        s0, s1 = max(0, lo), min(TW, lo + W * C)
                        if s1 > s0:
                            dst[h, p, s0:s1] = img[y, s0 - lo:s1 - lo]
            # weights: fp8e4 1.0 has bit pattern 0x38 (exp bias 7)
            ky, kxp, m = sp["ky"], sp["kxp"], sp["m"]
            cb = int(meta["col_base"][j])
            p = np.arange(128)[:, None]
            o = np.arange(112)[None, :]
            for g in range(G):
                rows = ky[kxp == m + g]
                if len(rows) == 0:
                    continue
                w0 = np.isin(p - o + 15, rows)
                w1 = np.isin(p - o - 1, rows)
                wt[:, cb + 224 * g:cb + 224 * g + 112][w0] = 0x38
                wt[:, cb + 224 * g + 112:cb + 224 * (g + 1)][w1] = 0x38
        in_maps.append({
            "ximg_sh": ximg_sh,
            "ximg_un": ximg_un,
            "wt": wt.view(ml_dtypes.float8_e4m3),
        })
    return meta, in_maps


# ---------------------------------------------------------------- device IR
WT_DTYPE_NAME = "float8e4"


def build_program(meta):
    import concourse.bacc as bacc
    from concourse.tile import TileContext

    bf16 = mybir.dt.bfloat16
    wdt = getattr(mybir.dt, WT_DTYPE_NAME)
    slots, modes, gmax = meta["slots"], meta["modes"], meta["gmax"]
    slot_sub = meta["slot_sub"]
    n_sh, n_un = len(meta["sh_idx"]), len(meta["un_idx"])

    nc = bacc.Bacc("TRN2")
    ximg_sh = nc.dram_tensor("ximg_sh", [max(n_sh, 1), 2, 128, meta["TWmax_sh"]],
                             bf16, kind="ExternalInput")
    ximg_un = nc.dram_tensor("ximg_un", [max(n_un, 1), 2, 128, meta["TWmax_un"]],
                             bf16, kind="ExternalInput")
    wt_d = nc.dram_tensor("wt", [128, meta["totcols"]], wdt, kind="ExternalInput")
    out_sh = nc.dram_tensor("out_sh", [max(n_sh, 1), H, W_SH], bf16,
                            kind="ExternalOutput")
    out_un = nc.dram_tensor("out_un", [max(n_un, 1), H, W_UN], bf16,
                            kind="ExternalOutput")

    totcols = meta["totcols"]
    n_slots = len(slots)

    with TileContext(nc) as tc:
        with tc.tile_pool(name="img", bufs=4) as ipool, \
             tc.tile_pool(name="wtp", bufs=3) as wpool, \
             tc.tile_pool(name="res", bufs=4) as rpool, \
             tc.tile_pool(name="ps0", bufs=2, space="PSUM") as pp00, \
             tc.tile_pool(name="ps1", bufs=2, space="PSUM") as pp01, \
             tc.tile_pool(name="ps2", bufs=2, space="PSUM") as pp10, \
             tc.tile_pool(name="ps3", bufs=2, space="PSUM") as pp11:
            psum_pools = [[pp00, pp01], [pp10, pp11]]
            for j in range(n_slots):
                sh = modes[j]
                G = gmax[j]
                Wp = W_SH if sh else W_UN
                TW = 3 * (G - 1) + Wp
                xsrc = ximg_sh if sh else ximg_un
                js = slot_sub[j]
                cb = int(meta["col_base"][j])
                # balance the two chunk chains so every matmul's column
                # count stays above the ~133ns LDWEIGHTS shadow (the
                # 512+160 split left the 160-chain weight-load-bound)
                half = Wp // 2 if Wp // 2 > 320 else CHUNK0
                chunks = [(0, half), (half, Wp - half)]

                t0 = ipool.tile([128, TW], bf16, tag="t0", name="t0")
                t1 = ipool.tile([128, TW], bf16, tag="t1", name="t1")
                nc.sync.dma_start(out=t0, in_=xsrc[js, 0, :, 0:TW])
                nc.sync.dma_start(out=t1, in_=xsrc[js, 1, :, 0:TW])
                wtt = wpool.tile([128, 224 * G], wdt, tag="wt", name="wtt")
                nc.sync.dma_start(out=wtt, in_=wt_d[:, cb:cb + 224 * G])

                tiles = [t0, t1]
                rts = []
                for hb in (0, 1):
                    rt = rpool.tile([112, Wp], bf16, tag=f"r{hb}", name=f"r{hb}")
                    rts.append(rt)
                    for ci, (c0, cw) in enumerate(chunks):
                        ps = psum_pools[hb][ci].tile(
                            [112, CHUNK0], mybir.dt.float32,
                            tag=f"ps{hb}{ci}", name=f"ps{hb}{ci}")
                        for g in range(G):
                            lhsT = wtt[:, 224 * g + 112 * hb:
                                       224 * g + 112 * hb + 112]
                            nc.tensor.matmul(ps[:, 0:cw], lhsT=lhsT,
                                             rhs=tiles[hb][:, 3 * g + c0:3 * g + c0 + cw],
                                             start=(g == 0), stop=(g == G - 1))
                        if ci == 0:
                            nc.scalar.activation(
                                out=rt[:, c0:c0 + cw], in_=ps[:, 0:cw],
                                func=mybir.ActivationFunctionType.Copy)
                        else:
                            nc.vector.tensor_copy(out=rt[:, c0:c0 + cw],
                                                  in_=ps[:, 0:cw])
                odst = out_sh if sh else out_un
                for hb in (0, 1):
                    nc.scalar.dma_start(out=odst[js, 112 * hb:112 * (hb + 1), :],
                                        in_=rts[hb])
    return nc


def run_cores(meta, in_maps, trace=False):
    from concourse.bass_utils import run_bass_kernel_spmd

    nc = build_program(meta)
    nc.compile()
    split_sync_waits(nc)
    res = run_bass_kernel_spmd(nc, in_maps, core_ids=list(range(len(in_maps))),
                               trace=trace)
    return res


# ------------------------------------------------------------------ unshard
def unshard(meta, results):
    slots, modes = meta["slots"], meta["modes"]
    samples = meta["samples"]
    slot_sub = meta["slot_sub"]
    B = sum(len(m) for m in slots)
    out = np.zeros((B, H, W, C), np.float32)
    r_idx = np.arange(H)
    # per-row pixel window start q0(r) = -k*r - E_h  (q = c - k*r - E_h)
    for c, r in enumerate(results):
        o_sh = np.asarray(r["out_sh"]).astype(np.float32)
        o_un = np.asarray(r["out_un"]).astype(np.float32)
        for j in range(len(slots)):
            b = slots[j][c]
            sp = samples[b]
            js = slot_sub[j]
            if modes[j]:
                arr = o_sh[js]  # [224, 1005]
                k, E = sp["k"], sp["E"]
                q0 = np.where(r_idx < 112,
                              -k * r_idx - E[0], -k * r_idx - E[1])
                cols = (3 * q0)[:, None] + np.arange(W * C)[None, :]
                img = np.take_along_axis(arr, cols, axis=1)
            else:
                img = o_un[js]
            img = img.reshape(H, W, C)
            if sp["tr"]:
                img = img.transpose(1, 0, 2)
            out[b] = img
    return out


def kernel(x, kernels_table, amt, angles):
    x = np.asarray(x, np.float32)
    kernels_table = np.asarray(kernels_table, np.float32)
    amt = np.asarray(amt)
    angles = np.asarray(angles)
    meta, in_maps = prepare_host(x, kernels_table, amt, angles)
    res = run_cores(meta, in_maps)
    return unshard(meta, res.results)
